# revision 17
# baseline (speedup 1.0000x reference)
"""Trainium2 Bass kernel for nn_AutocorrelationCorrelogram.

For nervegram [B=4, F=50, T=20000, C=2]: 300 periodic-Hann-windowed frames
of length 512 per (b,f,c) signal, circular autocorrelation via
Wiener-Khinchin (rfft -> |.|^2 -> irfft), relu, normalize by sqrt(zero
lag), keep 256 lags, mean over channels -> [4, 50, 300, 256].

Sharding: pure data parallel over the 200 (b,f) pairs -> 25 per core x 8
cores (SPMD, no collectives).

Kernel structure (per core, per superbatch of 20 frames x 25 bf):
  - DMA frames row-major [125 rows=(m,bf), 512t, 2c] (4KB contiguous rows)
  - PE-transpose to time-major yt[k] [128 t, 500 rows] per channel
  - rfft as matmuls with window folded into the DFT matrices; Wsin col 0
    carries the bin-256 cos column (sin col of bin 0 is identically zero)
  - P = Re^2 + Im^2 with row-0 fixups for the bin-256 trick
  - irfft matmuls use P *as the stationary operand* so the result lands
    as acf^T [rows, lags] (row-major for output DMA, per-partition norm);
    D is scaled by 0.25 so adding the two channels yields the channel
    mean of the normalized acf
  - norm: relu(acf * 1/sqrt(acf0 + 1e-30)) via ACT Sqrt + DVE reciprocal
    + ACT Relu with per-partition scale, then one DVE add for the
    channel mean.
"""

import sys

import numpy as np

sys.path.insert(0, "/opt/trn_rl_repo")

B, F, T, C = 4, 50, 20000, 2
NUM_FRAME = 300
LEN_FRAME = 512
LAGS = 256
NBINS = 257
N_CORES = 8
BF_PER_CORE = (B * F) // N_CORES  # 25

FRAMES_PER_SB = 20  # frames per superbatch
ROWS_PER_TILE = 125  # 5 frames x 25 bf
TILES_PER_SB = 4
N_SB_FULL = NUM_FRAME // FRAMES_PER_SB  # 15
NCOLS = 500  # rows per (c) group = 20*25

STARTS = np.linspace(0, T - LEN_FRAME, NUM_FRAME).astype(np.int64)


def build_weights():
    t = np.arange(LEN_FRAME, dtype=np.float64)
    w = 0.5 - 0.5 * np.cos(2.0 * np.pi * t / LEN_FRAME)  # periodic hann
    ang = 2.0 * np.pi * np.outer(t, np.arange(NBINS)) / LEN_FRAME
    Cm = np.cos(ang) * w[:, None]  # [512, 257]
    Sm = -np.sin(ang) * w[:, None]
    wcos = Cm[:, 0:256].reshape(4, 128, 256).copy()
    wsin = Sm[:, 0:256].reshape(4, 128, 256).copy()
    wsin[:, :, 0] = Cm[:, 256].reshape(4, 128)  # bin-256 cos column
    alpha = 0.25  # folds the channel-mean 0.5 (output scales with sqrt(alpha))
    k = np.arange(NBINS)
    coef = np.full(NBINS, 2.0)
    coef[0] = 1.0
    coef[256] = 1.0
    D = (alpha / LEN_FRAME) * coef[:, None] * np.cos(
        2.0 * np.pi * np.outer(k, np.arange(LAGS)) / LEN_FRAME
    )
    return (
        wcos.astype(np.float32),
        wsin.astype(np.float32),
        D.astype(np.float32),
        np.eye(128, dtype=np.float32),
    )


def build_nc(n_sb=N_SB_FULL, use_f32r=True, bf16_front=False):
    from contextlib import ExitStack

    import concourse.bacc as bacc
    import concourse.bass as bass
    import concourse.tile as tile
    from concourse import mybir

    f32 = mybir.dt.float32
    f32r = mybir.dt.float32r
    AF = mybir.ActivationFunctionType

    mmdt = f32r if use_f32r else f32
    bf16 = mybir.dt.bfloat16
    fdt = bf16 if bf16_front else f32  # frames/transpose dtype
    wdt = bf16 if bf16_front else mmdt  # rfft DFT-matrix dtype
    ytdt = bf16 if bf16_front else mmdt  # rfft moving-operand dtype

    nc = bacc.Bacc("TRN2", target_bir_lowering=False, debug=False)

    x = nc.dram_tensor("x", [BF_PER_CORE, T, C], f32, kind="ExternalInput").ap()
    wcos_d = nc.dram_tensor("wcos", [4, 128, 256], wdt, kind="ExternalInput").ap()
    wsin_d = nc.dram_tensor("wsin", [4, 128, 256], wdt, kind="ExternalInput").ap()
    dmat_d = nc.dram_tensor("dmat", [NBINS, LAGS], mmdt, kind="ExternalInput").ap()
    eye_d = nc.dram_tensor("eye", [128, 128], fdt, kind="ExternalInput").ap()
    out = nc.dram_tensor(
        "out", [BF_PER_CORE, NUM_FRAME, LAGS], f32, kind="ExternalOutput"
    ).ap()

    with tile.TileContext(nc) as tc, ExitStack() as ctx:
        consts = ctx.enter_context(tc.tile_pool(name="consts", bufs=1))
        sb_pool = ctx.enter_context(tc.tile_pool(name="work", bufs=1))
        pp = ctx.enter_context(tc.tile_pool(name="ps", bufs=1, space="PSUM"))

        # ---- load constants once ----
        wcos_sb = consts.tile([128, 4, 256], wdt, tag="wcos")
        wsin_sb = consts.tile([128, 4, 256], wdt, tag="wsin")
        for k in range(4):
            nc.sync.dma_start(out=wcos_sb[:, k, :], in_=wcos_d[k])
            nc.sync.dma_start(out=wsin_sb[:, k, :], in_=wsin_d[k])
        dm0 = consts.tile([128, 256], mmdt, tag="dm0")
        dm1 = consts.tile([128, 256], mmdt, tag="dm1")
        dm2 = consts.tile([1, 256], mmdt, tag="dm2")
        nc.sync.dma_start(out=dm0[:], in_=dmat_d[0:128])
        nc.sync.dma_start(out=dm1[:], in_=dmat_d[128:256])
        nc.sync.dma_start(out=dm2[:], in_=dmat_d[256:257])
        eye_sb = consts.tile([128, 128], fdt, tag="eye")
        nc.sync.dma_start(out=eye_sb[:], in_=eye_d[:])
        zero_b = consts.tile([128, 1], f32, tag="zerob")
        nc.vector.memset(zero_b[:], 0.0)
        eps_b = consts.tile([128, 1], f32, tag="epsb")
        nc.vector.memset(eps_b[:], 1e-30)

        def load_sb(s):
            m0 = s * FRAMES_PER_SB
            tiles = []
            for j in range(TILES_PER_SB):
                ft = sb_pool.tile(
                    [ROWS_PER_TILE, LEN_FRAME, C], fdt, tag="ft", bufs=12
                )
                # fold frames with equal start-stride into one DMA
                mm = 0
                while mm < 5:
                    m = m0 + 5 * j + mm
                    run = 1
                    while (
                        mm + run < 5
                        and STARTS[m + run] - STARTS[m + run - 1]
                        == STARTS[m + 1] - STARTS[m]
                    ):
                        run += 1
                    s0 = int(STARTS[m])
                    step = int(STARTS[m + 1] - STARTS[m]) if run > 1 else 0
                    src_ap = bass.AP(
                        tensor=x.tensor,
                        offset=x.offset + s0 * C,
                        ap=[
                            [step * C, run],
                            [T * C, BF_PER_CORE],
                            [C, LEN_FRAME],
                            [1, C],
                        ],
                    )
                    nc.gpsimd.dma_start(
                        out=ft[25 * mm : 25 * (mm + run)], in_=src_ap
                    )
                    mm += run
                tiles.append(ft)
            return tiles

        # prefetch frame loads 2 superbatches ahead so the gpsimd DMA
        # queue issues them before the current superbatch's tail work
        ft_queue = {}
        for s in range(min(2, n_sb)):
            ft_queue[s] = load_sb(s)

        for sb in range(n_sb):
            m0 = sb * FRAMES_PER_SB
            if sb + 2 < n_sb:
                ft_queue[sb + 2] = load_sb(sb + 2)
            ftiles = ft_queue.pop(sb)

            norm_c0 = []
            for c in range(C):
                # ---- transpose to time-major yt[k] = [128 t, 500 rows] ----
                yts = []
                for k in range(4):
                    if bf16_front:
                        # bf16 PSUM writes need 4B-aligned offsets: pad
                        # transpose groups to 128-col strides
                        trp = pp.tile([128, 4, 128], fdt, tag="tr", bufs=2)
                        for j in range(TILES_PER_SB):
                            nc.tensor.transpose(
                                trp[:, j, 0:125],
                                ftiles[j][:, 128 * k : 128 * k + 128, c : c + 1],
                                eye_sb[:125, :125],
                            )
                        yt = sb_pool.tile([128, NCOLS], ytdt, tag="yt", bufs=12)
                        nc.vector.tensor_copy(
                            yt.rearrange("p (j q) -> p j q", j=4),
                            trp[:, :, 0:125],
                        )
                    else:
                        trp = pp.tile([128, NCOLS], fdt, tag="tr", bufs=2)
                        for j in range(TILES_PER_SB):
                            nc.tensor.transpose(
                                trp[:, 125 * j : 125 * j + 125],
                                ftiles[j][:, 128 * k : 128 * k + 128, c : c + 1],
                                eye_sb[:125, :125],
                            )
                        yt = sb_pool.tile([128, NCOLS], ytdt, tag="yt", bufs=12)
                        nc.vector.tensor_copy(yt[:], trp[:])
                    yts.append(yt)

                # ---- rfft + P = Re^2 + Im^2, per half (short PSUM life) ----
                phs = []
                p256 = None
                for h in range(2):
                    rp = pp.tile([128, NCOLS], f32, tag="fft", bufs=4)
                    ip = pp.tile([128, NCOLS], f32, tag="fft", bufs=4)
                    for k in range(4):
                        nc.tensor.matmul(
                            rp[:],
                            wcos_sb[:, k, 128 * h : 128 * h + 128],
                            yts[k][:],
                            start=(k == 0),
                            stop=(k == 3),
                        )
                        nc.tensor.matmul(
                            ip[:],
                            wsin_sb[:, k, 128 * h : 128 * h + 128],
                            yts[k][:],
                            start=(k == 0),
                            stop=(k == 3),
                        )
                    sq_r = sb_pool.tile([128, NCOLS], f32, tag="sqr", bufs=3)
                    sq_i = sb_pool.tile([128, NCOLS], f32, tag="sqi", bufs=3)
                    nc.scalar.activation(sq_r[:], rp[:], AF.Square, bias=zero_b[:])
                    nc.scalar.activation(sq_i[:], ip[:], AF.Square, bias=zero_b[:])
                    if h == 0:
                        # P256 = Im_h0[0]^2 (Wsin_h0 col 0 carries cos-256)
                        p256 = sb_pool.tile([1, NCOLS], mmdt, tag="p256", bufs=3)
                        nc.scalar.activation(
                            p256[:], ip[0:1, :], AF.Square, bias=zero_b[0:1]
                        )
                    ph = sb_pool.tile([128, NCOLS], mmdt, tag=f"ph{h}", bufs=3)
                    nc.vector.tensor_add(ph[:], sq_r[:], sq_i[:])
                    if h == 0:
                        # fix P_h0[0] = Re_h0[0]^2 (undo the p256 slot)
                        nc.scalar.activation(
                            ph[0:1, :], rp[0:1, :], AF.Square, bias=zero_b[0:1]
                        )
                    phs.append(ph)

                # ---- irfft (P stationary) -> acf^T [125 rows, 256 lags] ----
                # norm stages batched 4-wide: all sqrts, then recips, then
                # relus, so the sqrt->recip->relu cross-engine chain never
                # serializes group-by-group
                acfps, sqcs, rccs = [], [], []
                for g in range(4):
                    acfp = pp.tile([ROWS_PER_TILE, LAGS], f32, tag="acf", bufs=2)
                    sl = slice(125 * g, 125 * g + 125)
                    nc.tensor.matmul(
                        acfp[:], phs[0][:, sl], dm0[:],
                        start=True, stop=False,
                    )
                    nc.tensor.matmul(
                        acfp[:], phs[1][:, sl], dm1[:],
                        start=False, stop=False,
                    )
                    nc.tensor.matmul(
                        acfp[:], p256[:, sl], dm2[:],
                        start=False, stop=True,
                    )
                    sqc = sb_pool.tile([ROWS_PER_TILE, 1], f32, tag="sqc", bufs=8)
                    nc.scalar.activation(
                        sqc[:], acfp[:, 0:1], AF.Sqrt, bias=eps_b[:125]
                    )
                    acfps.append(acfp)
                    sqcs.append(sqc)
                for g in range(4):
                    rcc = sb_pool.tile([ROWS_PER_TILE, 1], f32, tag="rcc", bufs=8)
                    nc.vector.reciprocal(out=rcc[:], in_=sqcs[g][:])
                    rccs.append(rcc)
                for g in range(4):
                    nt = sb_pool.tile(
                        [ROWS_PER_TILE, LAGS], f32, tag=f"nt{c}",
                        bufs=(8 if c == 0 else 3),
                    )
                    nc.scalar.activation(
                        nt[:], acfps[g][:], AF.Relu,
                        bias=zero_b[:125], scale=rccs[g][:],
                    )
                    if c == 0:
                        norm_c0.append(nt)
                    else:
                        # ---- channel mean (0.5 folded into D) + store ----
                        mt = sb_pool.tile(
                            [ROWS_PER_TILE, LAGS], f32, tag="mt", bufs=6
                        )
                        nc.gpsimd.tensor_add(mt[:], norm_c0[g][:], nt[:])
                        mf = m0 + 5 * g
                        nc.sync.dma_start(
                            out=out[:, mf : mf + 5, :].rearrange(
                                "bf mm l -> mm bf l"
                            ),
                            in_=mt[:],
                        )

    nc.compile()
    return nc


_NC_CACHE = {}


def _get_nc(n_sb=N_SB_FULL, use_f32r=True, bf16_front=False):
    key = (n_sb, use_f32r, bf16_front)
    if key not in _NC_CACHE:
        _NC_CACHE[key] = build_nc(n_sb, use_f32r, bf16_front)
    return _NC_CACHE[key]


def make_in_maps(nerv, bf16_front=False):
    import ml_dtypes

    xs = nerv.reshape(B * F, T, C)
    wcos, wsin, dmat, eye = build_weights()
    if bf16_front:
        wcos = wcos.astype(ml_dtypes.bfloat16)
        wsin = wsin.astype(ml_dtypes.bfloat16)
        eye = eye.astype(ml_dtypes.bfloat16)
    return [
        {
            "x": np.ascontiguousarray(xs[BF_PER_CORE * i : BF_PER_CORE * (i + 1)]),
            "wcos": wcos,
            "wsin": wsin,
            "dmat": dmat,
            "eye": eye,
        }
        for i in range(N_CORES)
    ]


def kernel(nervegram, trace=False, use_f32r=True, bf16_front=False):
    from concourse.bass_utils import run_bass_kernel_spmd

    nerv = np.ascontiguousarray(np.asarray(nervegram, dtype=np.float32))
    assert nerv.shape == (B, F, T, C)
    in_maps = make_in_maps(nerv, bf16_front)
    nc = _get_nc(use_f32r=use_f32r, bf16_front=bf16_front)
    res = run_bass_kernel_spmd(nc, in_maps, list(range(N_CORES)), trace=trace)
    full = np.concatenate([res.results[i]["out"] for i in range(N_CORES)], axis=0)
    out = full.reshape(B, F, NUM_FRAME, LAGS)
    if trace:
        return out, res
    return out


# revision 18
# speedup vs baseline: 1.3104x; 1.3104x over previous
"""Trainium2 Bass kernel for nn_AutocorrelationCorrelogram.

For nervegram [B=4, F=50, T=20000, C=2]: 300 periodic-Hann-windowed frames
of length 512 per (b,f,c) signal, circular autocorrelation via
Wiener-Khinchin (rfft -> |.|^2 -> irfft), relu, normalize by sqrt(zero
lag), keep 256 lags, mean over channels -> [4, 50, 300, 256].

Sharding: pure data parallel over the 200 (b,f) pairs -> 25 per core x 8
cores (SPMD, no collectives).

Kernel structure (per core, per superbatch of 20 frames x 25 bf):
  - DMA frames row-major [125 rows=(m,bf), 512t, 2c] (4KB contiguous rows)
  - PE-transpose to time-major yt[k] [128 t, 500 rows] per channel
  - rfft as matmuls with window folded into the DFT matrices; Wsin col 0
    carries the bin-256 cos column (sin col of bin 0 is identically zero)
  - P = Re^2 + Im^2 with row-0 fixups for the bin-256 trick
  - irfft matmuls use P *as the stationary operand* so the result lands
    as acf^T [rows, lags] (row-major for output DMA, per-partition norm);
    D is scaled by 0.25 so adding the two channels yields the channel
    mean of the normalized acf
  - norm: relu(acf * 1/sqrt(acf0 + 1e-30)) via ACT Sqrt + DVE reciprocal
    + ACT Relu with per-partition scale, then one DVE add for the
    channel mean.
"""

import sys

import numpy as np

sys.path.insert(0, "/opt/trn_rl_repo")

B, F, T, C = 4, 50, 20000, 2
NUM_FRAME = 300
LEN_FRAME = 512
LAGS = 256
NBINS = 257
N_CORES = 8
BF_PER_CORE = (B * F) // N_CORES  # 25

FRAMES_PER_SB = 20  # frames per superbatch
ROWS_PER_TILE = 125  # 5 frames x 25 bf
TILES_PER_SB = 4
N_SB_FULL = NUM_FRAME // FRAMES_PER_SB  # 15
NCOLS = 500  # rows per (c) group = 20*25

STARTS = np.linspace(0, T - LEN_FRAME, NUM_FRAME).astype(np.int64)


def build_weights():
    t = np.arange(LEN_FRAME, dtype=np.float64)
    w = 0.5 - 0.5 * np.cos(2.0 * np.pi * t / LEN_FRAME)  # periodic hann
    ang = 2.0 * np.pi * np.outer(t, np.arange(NBINS)) / LEN_FRAME
    Cm = np.cos(ang) * w[:, None]  # [512, 257]
    Sm = -np.sin(ang) * w[:, None]
    wcos = Cm[:, 0:256].reshape(4, 128, 256).copy()
    wsin = Sm[:, 0:256].reshape(4, 128, 256).copy()
    wsin[:, :, 0] = Cm[:, 256].reshape(4, 128)  # bin-256 cos column
    alpha = 0.25  # folds the channel-mean 0.5 (output scales with sqrt(alpha))
    k = np.arange(NBINS)
    coef = np.full(NBINS, 2.0)
    coef[0] = 1.0
    coef[256] = 1.0
    D = (alpha / LEN_FRAME) * coef[:, None] * np.cos(
        2.0 * np.pi * np.outer(k, np.arange(LAGS)) / LEN_FRAME
    )
    return (
        wcos.astype(np.float32),
        wsin.astype(np.float32),
        D.astype(np.float32),
        np.eye(128, dtype=np.float32),
    )


def build_nc(n_sb=N_SB_FULL, use_f32r=True, bf16_front=False):
    from contextlib import ExitStack

    import concourse.bacc as bacc
    import concourse.bass as bass
    import concourse.tile as tile
    from concourse import mybir

    f32 = mybir.dt.float32
    f32r = mybir.dt.float32r
    AF = mybir.ActivationFunctionType

    mmdt = f32r if use_f32r else f32
    bf16 = mybir.dt.bfloat16
    fdt = bf16 if bf16_front else f32  # frames/transpose dtype
    wdt = bf16 if bf16_front else mmdt  # rfft DFT-matrix dtype
    ytdt = bf16 if bf16_front else mmdt  # rfft moving-operand dtype

    nc = bacc.Bacc("TRN2", target_bir_lowering=False, debug=False)

    x = nc.dram_tensor("x", [BF_PER_CORE, T, C], f32, kind="ExternalInput").ap()
    wcos_d = nc.dram_tensor("wcos", [4, 128, 256], wdt, kind="ExternalInput").ap()
    wsin_d = nc.dram_tensor("wsin", [4, 128, 256], wdt, kind="ExternalInput").ap()
    dmat_d = nc.dram_tensor("dmat", [NBINS, LAGS], mmdt, kind="ExternalInput").ap()
    eye_d = nc.dram_tensor("eye", [128, 128], fdt, kind="ExternalInput").ap()
    out = nc.dram_tensor(
        "out", [BF_PER_CORE, NUM_FRAME, LAGS], f32, kind="ExternalOutput"
    ).ap()

    with tile.TileContext(nc) as tc, ExitStack() as ctx:
        consts = ctx.enter_context(tc.tile_pool(name="consts", bufs=1))
        sb_pool = ctx.enter_context(tc.tile_pool(name="work", bufs=1))
        pp = ctx.enter_context(tc.tile_pool(name="ps", bufs=1, space="PSUM"))

        # ---- load constants once ----
        wcos_sb = consts.tile([128, 4, 256], wdt, tag="wcos")
        wsin_sb = consts.tile([128, 4, 256], wdt, tag="wsin")
        for k in range(4):
            nc.sync.dma_start(out=wcos_sb[:, k, :], in_=wcos_d[k])
            nc.sync.dma_start(out=wsin_sb[:, k, :], in_=wsin_d[k])
        dm0 = consts.tile([128, 256], mmdt, tag="dm0")
        dm1 = consts.tile([128, 256], mmdt, tag="dm1")
        dm2 = consts.tile([1, 256], mmdt, tag="dm2")
        nc.sync.dma_start(out=dm0[:], in_=dmat_d[0:128])
        nc.sync.dma_start(out=dm1[:], in_=dmat_d[128:256])
        nc.sync.dma_start(out=dm2[:], in_=dmat_d[256:257])
        eye_sb = consts.tile([128, 128], fdt, tag="eye")
        nc.sync.dma_start(out=eye_sb[:], in_=eye_d[:])
        zero_b = consts.tile([128, 1], f32, tag="zerob")
        nc.vector.memset(zero_b[:], 0.0)
        eps_b = consts.tile([128, 1], f32, tag="epsb")
        nc.vector.memset(eps_b[:], 1e-30)

        def load_sb(s):
            m0 = s * FRAMES_PER_SB
            tiles = []
            for j in range(TILES_PER_SB):
                ft = sb_pool.tile(
                    [ROWS_PER_TILE, LEN_FRAME, C], fdt, tag="ft", bufs=12
                )
                # fold frames with equal start-stride into one DMA
                mm = 0
                while mm < 5:
                    m = m0 + 5 * j + mm
                    run = 1
                    while (
                        mm + run < 5
                        and STARTS[m + run] - STARTS[m + run - 1]
                        == STARTS[m + 1] - STARTS[m]
                    ):
                        run += 1
                    s0 = int(STARTS[m])
                    step = int(STARTS[m + 1] - STARTS[m]) if run > 1 else 0
                    src_ap = bass.AP(
                        tensor=x.tensor,
                        offset=x.offset + s0 * C,
                        ap=[
                            [step * C, run],
                            [T * C, BF_PER_CORE],
                            [C, LEN_FRAME],
                            [1, C],
                        ],
                    )
                    nc.gpsimd.dma_start(
                        out=ft[25 * mm : 25 * (mm + run)], in_=src_ap
                    )
                    mm += run
                tiles.append(ft)
            return tiles

        # prefetch frame loads 2 superbatches ahead so the gpsimd DMA
        # queue issues them before the current superbatch's tail work
        ft_queue = {}
        for s in range(min(2, n_sb)):
            ft_queue[s] = load_sb(s)

        for sb in range(n_sb):
            m0 = sb * FRAMES_PER_SB
            if sb + 2 < n_sb:
                ft_queue[sb + 2] = load_sb(sb + 2)
            ftiles = ft_queue.pop(sb)

            norm_c0 = []
            for c in range(C):
                # ---- transpose to time-major yt[k] = [128 t, 500 rows] ----
                yts = []
                for k in range(4):
                    if bf16_front:
                        # bf16 PSUM writes need 4B-aligned offsets: pad
                        # transpose groups to 128-col strides
                        trp = pp.tile([128, 4, 128], fdt, tag="tr", bufs=2)
                        for j in range(TILES_PER_SB):
                            nc.tensor.transpose(
                                trp[:, j, 0:125],
                                ftiles[j][:, 128 * k : 128 * k + 128, c : c + 1],
                                eye_sb[:125, :125],
                            )
                        yt = sb_pool.tile([128, NCOLS], ytdt, tag="yt", bufs=12)
                        nc.vector.tensor_copy(
                            yt.rearrange("p (j q) -> p j q", j=4),
                            trp[:, :, 0:125],
                        )
                    else:
                        trp = pp.tile([128, NCOLS], fdt, tag="tr", bufs=2)
                        for j in range(TILES_PER_SB):
                            nc.tensor.transpose(
                                trp[:, 125 * j : 125 * j + 125],
                                ftiles[j][:, 128 * k : 128 * k + 128, c : c + 1],
                                eye_sb[:125, :125],
                            )
                        yt = sb_pool.tile([128, NCOLS], ytdt, tag="yt", bufs=12)
                        nc.vector.tensor_copy(yt[:], trp[:])
                    yts.append(yt)

                # ---- rfft + P = Re^2 + Im^2, per half (short PSUM life) ----
                phs = []
                p256 = None
                for h in range(2):
                    rp = pp.tile([128, NCOLS], f32, tag="fft", bufs=4)
                    ip = pp.tile([128, NCOLS], f32, tag="fft", bufs=4)
                    for k in range(4):
                        nc.tensor.matmul(
                            rp[:],
                            wcos_sb[:, k, 128 * h : 128 * h + 128],
                            yts[k][:],
                            start=(k == 0),
                            stop=(k == 3),
                        )
                        nc.tensor.matmul(
                            ip[:],
                            wsin_sb[:, k, 128 * h : 128 * h + 128],
                            yts[k][:],
                            start=(k == 0),
                            stop=(k == 3),
                        )
                    sq_r = sb_pool.tile([128, NCOLS], f32, tag="sqr", bufs=3)
                    sq_i = sb_pool.tile([128, NCOLS], f32, tag="sqi", bufs=3)
                    nc.scalar.activation(sq_r[:], rp[:], AF.Square, bias=zero_b[:])
                    nc.scalar.activation(sq_i[:], ip[:], AF.Square, bias=zero_b[:])
                    if h == 0:
                        # P256 = Im_h0[0]^2 (Wsin_h0 col 0 carries cos-256)
                        p256 = sb_pool.tile([1, NCOLS], mmdt, tag="p256", bufs=3)
                        nc.scalar.activation(
                            p256[:], ip[0:1, :], AF.Square, bias=zero_b[0:1]
                        )
                    ph = sb_pool.tile([128, NCOLS], mmdt, tag=f"ph{h}", bufs=3)
                    nc.vector.tensor_add(ph[:], sq_r[:], sq_i[:])
                    if h == 0:
                        # fix P_h0[0] = Re_h0[0]^2 (undo the p256 slot)
                        nc.scalar.activation(
                            ph[0:1, :], rp[0:1, :], AF.Square, bias=zero_b[0:1]
                        )
                    phs.append(ph)

                # ---- irfft (P stationary) -> acf^T [125 rows, 256 lags] ----
                # norm stages batched 4-wide: all sqrts, then recips, then
                # relus, so the sqrt->recip->relu cross-engine chain never
                # serializes group-by-group
                acfps, sqcs, rccs = [], [], []
                for g in range(4):
                    acfp = pp.tile([ROWS_PER_TILE, LAGS], f32, tag="acf", bufs=2)
                    sl = slice(125 * g, 125 * g + 125)
                    nc.tensor.matmul(
                        acfp[:], phs[0][:, sl], dm0[:],
                        start=True, stop=False,
                    )
                    nc.tensor.matmul(
                        acfp[:], phs[1][:, sl], dm1[:],
                        start=False, stop=False,
                    )
                    nc.tensor.matmul(
                        acfp[:], p256[:, sl], dm2[:],
                        start=False, stop=True,
                    )
                    sqc = sb_pool.tile([ROWS_PER_TILE, 1], f32, tag="sqc", bufs=8)
                    nc.scalar.activation(
                        sqc[:], acfp[:, 0:1], AF.Sqrt, bias=eps_b[:125]
                    )
                    acfps.append(acfp)
                    sqcs.append(sqc)
                for g in range(4):
                    rcc = sb_pool.tile([ROWS_PER_TILE, 1], f32, tag="rcc", bufs=8)
                    nc.vector.reciprocal(out=rcc[:], in_=sqcs[g][:])
                    rccs.append(rcc)
                for g in range(4):
                    nt = sb_pool.tile(
                        [ROWS_PER_TILE, LAGS], f32, tag=f"nt{c}",
                        bufs=(8 if c == 0 else 3),
                    )
                    nc.scalar.activation(
                        nt[:], acfps[g][:], AF.Relu,
                        bias=zero_b[:125], scale=rccs[g][:],
                    )
                    if c == 0:
                        norm_c0.append(nt)
                    else:
                        # ---- channel mean (0.5 folded into D) + store ----
                        mt = sb_pool.tile(
                            [ROWS_PER_TILE, LAGS], f32, tag="mt", bufs=6
                        )
                        nc.vector.tensor_add(mt[:], norm_c0[g][:], nt[:])
                        mf = m0 + 5 * g
                        nc.gpsimd.dma_start(
                            out=out[:, mf : mf + 5, :].rearrange(
                                "bf mm l -> mm bf l"
                            ),
                            in_=mt[:],
                        )

    nc.compile()
    return nc


_NC_CACHE = {}


def _get_nc(n_sb=N_SB_FULL, use_f32r=True, bf16_front=False):
    key = (n_sb, use_f32r, bf16_front)
    if key not in _NC_CACHE:
        _NC_CACHE[key] = build_nc(n_sb, use_f32r, bf16_front)
    return _NC_CACHE[key]


def make_in_maps(nerv, bf16_front=False):
    import ml_dtypes

    xs = nerv.reshape(B * F, T, C)
    wcos, wsin, dmat, eye = build_weights()
    if bf16_front:
        wcos = wcos.astype(ml_dtypes.bfloat16)
        wsin = wsin.astype(ml_dtypes.bfloat16)
        eye = eye.astype(ml_dtypes.bfloat16)
    return [
        {
            "x": np.ascontiguousarray(xs[BF_PER_CORE * i : BF_PER_CORE * (i + 1)]),
            "wcos": wcos,
            "wsin": wsin,
            "dmat": dmat,
            "eye": eye,
        }
        for i in range(N_CORES)
    ]


def kernel(nervegram, trace=False, use_f32r=True, bf16_front=False):
    from concourse.bass_utils import run_bass_kernel_spmd

    nerv = np.ascontiguousarray(np.asarray(nervegram, dtype=np.float32))
    assert nerv.shape == (B, F, T, C)
    in_maps = make_in_maps(nerv, bf16_front)
    nc = _get_nc(use_f32r=use_f32r, bf16_front=bf16_front)
    res = run_bass_kernel_spmd(nc, in_maps, list(range(N_CORES)), trace=trace)
    full = np.concatenate([res.results[i]["out"] for i in range(N_CORES)], axis=0)
    out = full.reshape(B, F, NUM_FRAME, LAGS)
    if trace:
        return out, res
    return out
